# revision 1
# baseline (speedup 1.0000x reference)
"""Bass/Trainium2 kernel for nn_EnhancedOrthogonal (complex column-orthogonalization).

Full inputs in, full outputs out. Internally: shard the M=16384 rows across 8
NeuronCores (2048 rows each). Per iteration the per-shard partial Gram matrix
Q^H Q (R x R complex) is AllReduced across cores; the column-normalization is
folded into the correction matrix B = D^-1 (I - 0.5*offdiag(D^-1 G' D^-1)) so
each iteration is exactly two complex matmuls per shard:
    G' = Q'^H Q'   (AllReduce over cores)
    Q'_next = Q' @ B(G')
with a final column rescale from one extra (diagonal-blocks-only) Gram pass.
"""

from contextlib import ExitStack

import numpy as np

import concourse.bacc as bacc
import concourse.bass as bass
import concourse.mybir as mybir
import concourse.tile as tile
from concourse.bass import ds, ts
from concourse.bass_utils import run_bass_kernel_spmd
from concourse.masks import make_identity

P = 128
M_FULL = 16384
R = 1024
N_CORES = 8
MS = M_FULL // N_CORES          # 2048 rows per core
NMT = MS // P                   # 16 m-tiles per core
NAB = R // P                    # 8 column blocks of 128
NCH = 2                         # 512-wide column chunks
CW = R // NCH                   # 512
EPS = 1e-8
F32 = mybir.dt.float32

_CACHE = {}


def _finalize(nc):
    nc.compile()
    # Tile's deferred wait assignment can leave multi-wait DMAs that the
    # in-compile generate_event_semaphores pass missed; DMA instructions
    # support a single HW wait slot, so re-split (and re-codegen) here.
    nc.generate_event_semaphores()
    nc.codegen_inst_isa_subclasses()


def _build_nc(debug_stage: int = 99, reps: int = 1, single_core: bool = False):
    nc = bacc.Bacc("TRN2", target_bir_lowering=False, debug=False,
                   num_devices=1 if single_core else N_CORES)
    x = nc.dram_tensor("x", [MS, R, 2], F32, kind="ExternalInput")
    out = nc.dram_tensor("out", [MS, R, 2], F32, kind="ExternalOutput")

    with tile.TileContext(nc) as tc, ExitStack() as ctx:
        consts = ctx.enter_context(tc.tile_pool(name="consts", bufs=1))
        persist = ctx.enter_context(tc.tile_pool(name="persist", bufs=1))
        dram = ctx.enter_context(tc.tile_pool(name="dram", bufs=1, space="DRAM"))

        identity = consts.tile([P, P], F32)
        make_identity(nc, identity)
        omI = consts.tile([P, P], F32)  # 1 - I
        nc.vector.tensor_scalar(out=omI, in0=identity, scalar1=-1.0,
                                scalar2=1.0, op0=mybir.AluOpType.mult,
                                op1=mybir.AluOpType.add)
        ones1 = consts.tile([1, P], F32)
        nc.vector.memset(ones1, 1.0)

        def bcast_row(row, dst, psum_pool, tag):
            """dst[p, :] = row[0, :] for all p, via K=1 outer-product matmul."""
            for n in range(NCH):
                v_ps = psum_pool.tile([P, CW], F32, tag=tag, name=tag)
                nc.tensor.matmul(v_ps, ones1, row[:, ds(n * CW, CW)],
                                 start=True, stop=True)
                nc.vector.tensor_copy(dst[:, ds(n * CW, CW)], v_ps)

        # DRAM scratch ([m, comp, r] so one m-block is a single 1MB DMA)
        q1 = dram.tile([MS, 2, R], F32, tag="q1", name="q1")
        q2 = dram.tile([MS, 2, R], F32, tag="q2", name="q2")

        # persistent SBUF: the B matrix (complex, [R, R] as 8 row-blocks
        # each) plus the Karatsuba sum B_re + B_im.
        b_t = [[persist.tile([P, R], F32, tag=f"b{c}_{t}", name=f"b{c}_{t}")
                for t in range(NAB)] for c in range(3)]

        # ---------------- gram pass ----------------
        def gram_pass(load_cols, gram_dst):
            """G' = Q'^H Q' partials -> gram_dst DRAM.

            G_re = Qr^T Qr + Qi^T Qi directly; G_im = T - T^T with a single
            product T = Qr^T Qi (saves 256 matmuls, costs 64 PE transposes).

            load_cols(pool, m, ch) -> (qr, qi) SBUF [P, CW] tiles holding
            Q'[m-block, ch*CW:(ch+1)*CW] for each component.
            """
            with (
                tc.tile_pool(name="gr_ld", bufs=3) as ldp,
                tc.tile_pool(name="gr_ps", bufs=1, space="PSUM") as psp,
                tc.tile_pool(name="gr_out", bufs=3) as gop,
                tc.tile_pool(name="gr_T", bufs=1) as tsp,
            ):
                t_sb = [tsp.tile([P, R], F32, tag=f"Tsb{a}", name=f"Tsb{a}")
                        for a in range(NAB)]
                # Phase A: G_re, 8 psum tiles per rhs chunk
                for nch in range(NCH):
                    ps = [psp.tile([P, CW], F32, tag=f"gre{t}",
                                   name=f"gre{t}") for t in range(NAB)]
                    for m in range(NMT):
                        qr0, qi0 = load_cols(ldp, m, 0)
                        qr1, qi1 = load_cols(ldp, m, 1)
                        qr_r, qi_r = (qr0, qi0) if nch == 0 else (qr1, qi1)
                        first, last = m == 0, m == NMT - 1
                        for a in range(NAB):
                            qr_l = qr0 if a < 4 else qr1
                            qi_l = qi0 if a < 4 else qi1
                            sl = ds((a % 4) * P, P)
                            nc.tensor.matmul(ps[a], qr_l[:, sl], qr_r,
                                             start=first, stop=False)
                            nc.tensor.matmul(ps[a], qi_l[:, sl], qi_r,
                                             start=False, stop=last)
                    for a in range(NAB):
                        g_sb = gop.tile([P, CW], F32, tag="gsb")
                        nc.vector.tensor_copy(g_sb, ps[a])
                        nc.sync.dma_start(
                            gram_dst[0, ts(a, P), ds(nch * CW, CW)], g_sb)
                # Phase B: T = Qr^T Qi, 8 psum tiles per rhs chunk
                for nch in range(NCH):
                    ps = [psp.tile([P, CW], F32, tag=f"gre{t}",
                                   name=f"gre{t}") for t in range(NAB)]
                    for m in range(NMT):
                        qr0, _ = load_cols(ldp, m, 0)
                        qr1, qi1 = load_cols(ldp, m, 1)
                        if nch == 0:
                            _, qi_r = load_cols(ldp, m, 0)
                        else:
                            qi_r = qi1
                        first, last = m == 0, m == NMT - 1
                        for a in range(NAB):
                            qr_l = qr0 if a < 4 else qr1
                            sl = ds((a % 4) * P, P)
                            nc.tensor.matmul(ps[a], qr_l[:, sl], qi_r,
                                             start=first, stop=last)
                    for a in range(NAB):
                        nc.vector.tensor_copy(t_sb[a][:, ds(nch * CW, CW)],
                                              ps[a])
                # Phase C: G_im tiles = T - T^T
                for a in range(NAB):
                    for g in range(NCH):
                        tp = psp.tile([P, 4, P], F32,
                                      tag=f"gre{(2 * a + g) % NAB}",
                                      name="gimtr")
                        for k in range(4):
                            b = 4 * g + k
                            nc.tensor.transpose(tp[:, k, :],
                                                t_sb[b][:, ts(a, P)],
                                                identity)
                        gim = gop.tile([P, 4, P], F32, tag="gimsb")
                        tsl = t_sb[a][:, ds(g * CW, CW)].rearrange(
                            "p (b k) -> p b k", b=4)
                        nc.vector.tensor_sub(gim, tsl, tp)
                        nc.sync.dma_start(
                            gram_dst[1, ts(a, P), ds(g * CW, CW)], gim)

        # ---------------- B build ----------------
        def build_b(gram_src):
            with (
                tc.tile_pool(name="bb", bufs=2) as bp,
                tc.tile_pool(name="bb_ps", bufs=2, space="PSUM") as bpp,
            ):
                dsq = bp.tile([P, NAB], F32, tag="bb_dsq")
                for t in range(NAB):
                    gd = bp.tile([P, P], F32, tag="bb_gd")
                    nc.sync.dma_start(gd, gram_src[0, ts(t, P), ts(t, P)])
                    nc.vector.tensor_mul(gd, gd, identity)
                    nc.vector.tensor_reduce(
                        dsq[:, ds(t, 1)], gd, mybir.AxisListType.X,
                        mybir.AluOpType.add)
                ninv2 = bp.tile([P, NAB], F32, tag="bb_ninv2")
                ninv = bp.tile([P, NAB], F32, tag="bb_ninv")
                w = bp.tile([P, NAB], F32, tag="bb_w")
                nc.vector.tensor_scalar_add(ninv2, dsq, EPS)
                nc.vector.reciprocal(ninv2, ninv2)
                nc.scalar.sqrt(ninv, ninv2)
                nc.vector.tensor_scalar_mul(w, ninv2, -0.5)
                # vrow [1, R]: ninv flattened in column order
                vT_ps = bpp.tile([NAB, P], F32, tag="bb_vT")
                nc.tensor.transpose(vT_ps, ninv, identity)
                vT = bp.tile([NAB, P], F32, tag="bb_vTs")
                nc.vector.tensor_copy(vT, vT_ps)
                vrow = persist.tile([1, R], F32, tag="vrow", name="vrow")
                nc.sync.dma_start(vrow, vT)
                vfull = bp.tile([P, R], F32, tag="bb_vfull")
                bcast_row(vrow, vfull, bpp, "bb_vps")
                for c in range(2):
                    for t in range(NAB):
                        b = b_t[c][t]
                        nc.sync.dma_start(b, gram_src[c, ts(t, P), :])
                        nc.vector.tensor_scalar_mul(b, b, w[:, ds(t, 1)])
                        nc.vector.tensor_mul(b, b, vfull)
                        bd = b[:, ts(t, P)]
                        nc.vector.tensor_mul(bd, bd, omI)
                        if c == 0:
                            dg = bp.tile([P, P], F32, tag="bb_dg")
                            nc.vector.tensor_scalar_mul(dg, identity,
                                                        ninv[:, ds(t, 1)])
                            nc.vector.tensor_add(bd, bd, dg)
                for t in range(NAB):
                    nc.vector.tensor_add(b_t[2][t], b_t[0][t], b_t[1][t])

        # ---------------- update pass ----------------
        def update_pass(load_rows, q_dst, acc_sq):
            """Q_next[m] = Q'[m] @ B via Karatsuba (3 real matmul products);
            optionally accumulate per-column sum of squares into acc_sq [P, R]
            (still needs a cross-partition reduce afterwards)."""
            with (
                tc.tile_pool(name="up_ld", bufs=2) as ldp,
                tc.tile_pool(name="up_t", bufs=2) as tp_sb,
                tc.tile_pool(name="up_ps", bufs=2, space="PSUM") as tpp,
                tc.tile_pool(name="up_ops", bufs=1, space="PSUM") as opp,
                tc.tile_pool(name="up_out", bufs=2) as outp,
            ):
                for m in range(NMT):
                    qr_t, qi_t = load_rows(ldp, m)
                    qrT = tp_sb.tile([P, NAB, P], F32, tag="qrT")
                    qiT = tp_sb.tile([P, NAB, P], F32, tag="qiT")
                    qsT = tp_sb.tile([P, NAB, P], F32, tag="qsT")
                    for src, dstT in ((qr_t, qrT), (qi_t, qiT)):
                        for g in range(2):
                            tp = tpp.tile([P, 4, P], F32, tag="tp", name="tp")
                            for k in range(4):
                                nc.tensor.transpose(tp[:, k, :],
                                                    src[:, ts(4 * g + k, P)],
                                                    identity)
                            nc.scalar.copy(dstT[:, ds(4 * g, 4), :], tp)
                    nc.vector.tensor_add(qsT, qrT, qiT)
                    p1 = opp.tile([P, R], F32, tag="p1", name="p1")
                    p2 = opp.tile([P, R], F32, tag="p2", name="p2")
                    p3 = opp.tile([P, R], F32, tag="p3", name="p3")
                    for n in range(NCH):
                        nsl = ds(n * CW, CW)
                        for ps, qT, bc in ((p1, qrT, 0), (p2, qiT, 1),
                                           (p3, qsT, 2)):
                            for a in range(NAB):
                                nc.tensor.matmul(ps[:, nsl], qT[:, a, :],
                                                 b_t[bc][a][:, nsl],
                                                 start=(a == 0),
                                                 stop=(a == NAB - 1))
                    qn = outp.tile([P, 2, R], F32, tag="qn", name="qn")
                    nc.vector.tensor_copy(qn[:, 0, :], p1)
                    nc.scalar.copy(qn[:, 1, :], p3)
                    nc.vector.tensor_sub(qn[:, 1, :], qn[:, 1, :], p1)
                    nc.vector.tensor_sub(qn[:, 1, :], qn[:, 1, :], p2)
                    nc.vector.tensor_sub(qn[:, 0, :], qn[:, 0, :], p2)
                    if acc_sq is not None:
                        sq = ldp.tile([P, 2, R], F32, tag="sq", name="sq")
                        nc.vector.tensor_mul(sq, qn, qn)
                        if m == 0:
                            nc.vector.tensor_add(acc_sq, sq[:, 0, :],
                                                 sq[:, 1, :])
                        else:
                            nc.vector.tensor_add(acc_sq, acc_sq, sq[:, 0, :])
                            nc.vector.tensor_add(acc_sq, acc_sq, sq[:, 1, :])
                    nc.sync.dma_start(q_dst[ts(m, P)], qn)

        # ---------------- loaders ----------------
        def load_cols_x(pool, m, ch):
            xt = pool.tile([P, CW, 2], F32, tag="xcols")
            nc.sync.dma_start(xt, x[ts(m, P), ds(ch * CW, CW), :])
            qr = pool.tile([P, CW], F32, tag="xc_r")
            qi = pool.tile([P, CW], F32, tag="xc_i")
            nc.vector.tensor_copy(qr, xt[:, :, 0])
            nc.vector.tensor_copy(qi, xt[:, :, 1])
            return qr, qi

        def load_cols_q1(pool, m, ch):
            qt = pool.tile([P, 2, CW], F32, tag="xcols")
            nc.sync.dma_start(qt, q1[ts(m, P), :, ds(ch * CW, CW)])
            return qt[:, 0, :], qt[:, 1, :]

        def load_rows_x(pool, m):
            xt = pool.tile([P, R, 2], F32, tag="xrows")
            nc.sync.dma_start(xt, x[ts(m, P), :, :])
            qr = pool.tile([P, R], F32, tag="xr_r")
            qi = pool.tile([P, R], F32, tag="xr_i")
            nc.vector.tensor_copy(qr, xt[:, :, 0])
            nc.vector.tensor_copy(qi, xt[:, :, 1])
            return qr, qi

        def load_rows_q1(pool, m):
            qt = pool.tile([P, 2, R], F32, tag="xrows")
            nc.sync.dma_start(qt, q1[ts(m, P)])
            return qt[:, 0, :], qt[:, 1, :]

        rg = [list(range(N_CORES))]

        def all_reduce(dst, src):
            if single_core:
                nc.sync.dma_start(dst[:], src[:])
            else:
                nc.gpsimd.collective_compute(
                    "AllReduce", mybir.AluOpType.add, replica_groups=rg,
                    ins=[src[:]], outs=[dst[:]])

        def debug_out():
            """Write gram_in[0] head into out so every stage has output."""
            with tc.tile_pool(name="dbg", bufs=2) as dp:
                for m in range(NMT):
                    t = dp.tile([P, R, 2], F32, tag="dbg_t")
                    nc.vector.memset(t, 0.0)
                    nc.sync.dma_start(out[ts(m, P), :, :], t)

        def _one_rep(rep):
            gram_in = [dram.tile([2, R, R], F32, tag=f"gin{k}_{rep}",
                                 name=f"gin{k}_{rep}") for k in range(2)]
            gram_out = [dram.tile([2, R, R], F32, tag=f"gout{k}_{rep}",
                                  name=f"gout{k}_{rep}", addr_space="Shared")
                        for k in range(2)]
            dsq_in = dram.tile([P, NAB], F32, tag=f"dsq_in_{rep}",
                               name=f"dsq_in_{rep}")
            dsq_out = dram.tile([P, NAB], F32, tag=f"dsq_out_{rep}",
                                name=f"dsq_out_{rep}", addr_space="Shared")
            gram_pass(load_cols_x, gram_in[0])
            if debug_stage >= 2:
                all_reduce(gram_out[0], gram_in[0])
            if debug_stage >= 3:
                build_b(gram_out[0])
            if debug_stage >= 4:
                update_pass(load_rows_x, q1, None)

            # ======== iteration 1 ========
            if debug_stage >= 5:
                gram_pass(load_cols_q1, gram_in[1])
                all_reduce(gram_out[1], gram_in[1])
                build_b(gram_out[1])
            def _full_tail():
                acc_sq = persist.tile([P, R], F32, tag="acc_sq",
                                      name="acc_sq")
                update_pass(load_rows_q1, q2, acc_sq)

                # ======== final column norms ========
                with (
                    tc.tile_pool(name="fn", bufs=2) as fp,
                    tc.tile_pool(name="fn_ps", bufs=2, space="PSUM") as fpp,
                ):
                    # cross-partition reduce of acc_sq via ones-matmul
                    dsq_row = fp.tile([1, R], F32, tag="fn_dsqrow")
                    onesc = fp.tile([P, 1], F32, tag="fn_onesc")
                    nc.vector.memset(onesc, 1.0)
                    for n in range(NCH):
                        nsl = ds(n * CW, CW)
                        rp = fpp.tile([1, CW], F32, tag="fn_rp", name="fn_rp")
                        nc.tensor.matmul(rp, onesc, acc_sq[:, nsl],
                                         start=True, stop=True)
                        nc.vector.tensor_copy(dsq_row[:, nsl], rp)
                    nc.sync.dma_start(dsq_in, dsq_row)
                    all_reduce(dsq_out, dsq_in)
                    srow = fp.tile([1, R], F32, tag="fn_srow")
                    nc.sync.dma_start(srow, dsq_out)
                    nc.vector.tensor_scalar_add(srow, srow, EPS)
                    nc.vector.reciprocal(srow, srow)
                    nc.scalar.sqrt(srow, srow)
                    sfull = persist.tile([P, 2, R], F32, tag="sfull",
                                         name="sfull")
                    for n in range(NCH):
                        nsl = ds(n * CW, CW)
                        v_ps = fpp.tile([P, CW], F32, tag="fn_sps",
                                        name="fn_sps")
                        nc.tensor.matmul(v_ps, ones1, srow[:, nsl],
                                         start=True, stop=True)
                        nc.vector.tensor_copy(sfull[:, 0, nsl], v_ps)
                        nc.vector.tensor_copy(sfull[:, 1, nsl], v_ps)

                # ======== final scale + interleave store ========
                with tc.tile_pool(name="st", bufs=3) as sp:
                    for m in range(NMT):
                        qt = sp.tile([P, 2, R], F32, tag="st_q")
                        nc.sync.dma_start(qt, q2[ts(m, P)])
                        nc.vector.tensor_mul(qt, qt, sfull)
                        ot = sp.tile([P, R, 2], F32, tag="st_o")
                        nc.vector.tensor_copy(ot[:, :, 0], qt[:, 0, :])
                        nc.vector.tensor_copy(ot[:, :, 1], qt[:, 1, :])
                        nc.sync.dma_start(out[ts(m, P), :, :], ot)

            if debug_stage < 6:
                debug_out()
            else:
                _full_tail()

        for _rep in range(reps):
            _one_rep(_rep)

    _finalize(nc)
    return nc


def kernel(x: np.ndarray) -> np.ndarray:
    assert x.shape == (M_FULL, R, 2) and x.dtype == np.float32
    if "nc" not in _CACHE:
        _CACHE["nc"] = _build_nc()
    nc = _CACHE["nc"]
    in_maps = [{"x": np.ascontiguousarray(x[i * MS:(i + 1) * MS])}
               for i in range(N_CORES)]
    res = run_bass_kernel_spmd(nc, in_maps, core_ids=list(range(N_CORES)))
    return np.concatenate([res.results[i]["out"] for i in range(N_CORES)],
                          axis=0)



# revision 7
# speedup vs baseline: 1.1354x; 1.1354x over previous
"""Bass/Trainium2 kernel for nn_EnhancedOrthogonal (complex column-orthogonalization).

Full inputs in, full outputs out. Internally: shard the M=16384 rows across 8
NeuronCores (2048 rows each). Per iteration the per-shard partial Gram matrix
Q^H Q (R x R complex) is AllReduced across cores; the column-normalization is
folded into the correction matrix B = D^-1 (I - 0.5*offdiag(D^-1 G' D^-1)) so
each iteration is exactly two complex matmuls per shard:
    G' = Q'^H Q'   (AllReduce over cores)
    Q'_next = Q' @ B(G')
with a final column rescale from one extra (diagonal-blocks-only) Gram pass.
"""

from contextlib import ExitStack

import numpy as np

import concourse.bacc as bacc
import concourse.bass as bass
import concourse.mybir as mybir
import concourse.tile as tile
from concourse.bass import ds, ts
from concourse.bass_utils import run_bass_kernel_spmd
from concourse.masks import make_identity

P = 128
M_FULL = 16384
R = 1024
N_CORES = 8
MS = M_FULL // N_CORES          # 2048 rows per core
NMT = MS // P                   # 16 m-tiles per core
NAB = R // P                    # 8 column blocks of 128
NCH = 2                         # 512-wide column chunks
CW = R // NCH                   # 512
EPS = 1e-8
F32 = mybir.dt.float32
BF16 = mybir.dt.bfloat16

_CACHE = {}


def _finalize(nc):
    nc.compile()
    # Tile's deferred wait assignment can leave multi-wait DMAs that the
    # in-compile generate_event_semaphores pass missed; DMA instructions
    # support a single HW wait slot, so re-split (and re-codegen) here.
    nc.generate_event_semaphores()
    nc.codegen_inst_isa_subclasses()


def _build_nc(debug_stage: int = 99, reps: int = 1, single_core: bool = False):
    nc = bacc.Bacc("TRN2", target_bir_lowering=False, debug=False,
                   num_devices=1 if single_core else N_CORES)
    x = nc.dram_tensor("x", [MS, R, 2], F32, kind="ExternalInput")
    out = nc.dram_tensor("out", [MS, R, 2], F32, kind="ExternalOutput")

    with tile.TileContext(nc) as tc, ExitStack() as ctx:
        consts = ctx.enter_context(tc.tile_pool(name="consts", bufs=1))
        persist = ctx.enter_context(tc.tile_pool(name="persist", bufs=1))
        dram = ctx.enter_context(tc.tile_pool(name="dram", bufs=1, space="DRAM"))

        identity = consts.tile([P, P], F32)
        make_identity(nc, identity)
        identity_bf = consts.tile([P, P], BF16)
        nc.vector.tensor_copy(identity_bf, identity)
        omI = consts.tile([P, P], F32)  # 1 - I
        nc.vector.tensor_scalar(out=omI, in0=identity, scalar1=-1.0,
                                scalar2=1.0, op0=mybir.AluOpType.mult,
                                op1=mybir.AluOpType.add)
        ones1 = consts.tile([1, P], F32)
        nc.vector.memset(ones1, 1.0)

        def bcast_row(row, dst, psum_pool, tag):
            """dst[p, :] = row[0, :] for all p, via K=1 outer-product matmul."""
            for n in range(NCH):
                v_ps = psum_pool.tile([P, CW], F32, tag=tag, name=tag)
                nc.tensor.matmul(v_ps, ones1, row[:, ds(n * CW, CW)],
                                 start=True, stop=True)
                nc.vector.tensor_copy(dst[:, ds(n * CW, CW)], v_ps)

        # DRAM scratch ([m, comp, r] so one m-block is a single 1MB DMA)
        q1 = dram.tile([MS, 2, R], BF16, tag="q1", name="q1")
        q2 = dram.tile([MS, 2, R], BF16, tag="q2", name="q2")

        # persistent SBUF: the B matrix (complex, [R, R] as 8 row-blocks
        # each) plus the Karatsuba sum B_re + B_im.
        b_t = [[persist.tile([P, R], BF16, tag=f"b{c}_{t}",
                              name=f"b{c}_{t}")
                for t in range(NAB)] for c in range(3)]

        # ---------------- gram pass ----------------
        def gram_pass(load_cols, gram_dst):
            """G' = Q'^H Q' partials -> gram_dst DRAM.

            G_re = Qr^T Qr + Qi^T Qi directly; G_im = T - T^T with a single
            product T = Qr^T Qi (saves 256 matmuls, costs 64 PE transposes).

            load_cols(pool, m, ch) -> (qr, qi) SBUF [P, CW] tiles holding
            Q'[m-block, ch*CW:(ch+1)*CW] for each component.
            """
            with (
                tc.tile_pool(name="gr_ld", bufs=3) as ldp,
                tc.tile_pool(name="gr_ps", bufs=1, space="PSUM") as psp,
                tc.tile_pool(name="gr_out", bufs=3) as gop,
                tc.tile_pool(name="gr_T", bufs=1) as tsp,
            ):
                t_sb = [tsp.tile([P, R], F32, tag=f"Tsb{a}", name=f"Tsb{a}")
                        for a in range(NAB)]
                # Phase A: G_re, 8 psum tiles per rhs chunk
                for nch in range(NCH):
                    ps = [psp.tile([P, CW], F32, tag=f"gre{t}",
                                   name=f"gre{t}") for t in range(NAB)]
                    for m in range(NMT):
                        qr0, qi0 = load_cols(ldp, m, 0)
                        qr1, qi1 = load_cols(ldp, m, 1)
                        qr_r, qi_r = (qr0, qi0) if nch == 0 else (qr1, qi1)
                        first, last = m == 0, m == NMT - 1
                        for a in range(NAB):
                            qr_l = qr0 if a < 4 else qr1
                            qi_l = qi0 if a < 4 else qi1
                            sl = ds((a % 4) * P, P)
                            nc.tensor.matmul(ps[a], qr_l[:, sl], qr_r,
                                             start=first, stop=False)
                            nc.tensor.matmul(ps[a], qi_l[:, sl], qi_r,
                                             start=False, stop=last)
                    for a in range(NAB):
                        g_sb = gop.tile([P, CW], F32, tag="gsb")
                        nc.vector.tensor_copy(g_sb, ps[a])
                        nc.sync.dma_start(
                            gram_dst[0, ts(a, P), ds(nch * CW, CW)], g_sb)
                # Phase B: T = Qr^T Qi, 8 psum tiles per rhs chunk
                for nch in range(NCH):
                    ps = [psp.tile([P, CW], F32, tag=f"gre{t}",
                                   name=f"gre{t}") for t in range(NAB)]
                    for m in range(NMT):
                        qr0, _ = load_cols(ldp, m, 0)
                        qr1, qi1 = load_cols(ldp, m, 1)
                        if nch == 0:
                            _, qi_r = load_cols(ldp, m, 0)
                        else:
                            qi_r = qi1
                        first, last = m == 0, m == NMT - 1
                        for a in range(NAB):
                            qr_l = qr0 if a < 4 else qr1
                            sl = ds((a % 4) * P, P)
                            nc.tensor.matmul(ps[a], qr_l[:, sl], qi_r,
                                             start=first, stop=last)
                    for a in range(NAB):
                        nc.vector.tensor_copy(t_sb[a][:, ds(nch * CW, CW)],
                                              ps[a])
                # Phase C: G_im tiles = T - T^T
                for a in range(NAB):
                    for g in range(NCH):
                        tp = psp.tile([P, 4, P], F32,
                                      tag=f"gre{(2 * a + g) % NAB}",
                                      name="gimtr")
                        for k in range(4):
                            b = 4 * g + k
                            nc.tensor.transpose(tp[:, k, :],
                                                t_sb[b][:, ts(a, P)],
                                                identity)
                        gim = gop.tile([P, 4, P], F32, tag="gimsb")
                        tsl = t_sb[a][:, ds(g * CW, CW)].rearrange(
                            "p (b k) -> p b k", b=4)
                        nc.vector.tensor_sub(gim, tsl, tp)
                        nc.sync.dma_start(
                            gram_dst[1, ts(a, P), ds(g * CW, CW)], gim)

        # ---------------- B build ----------------
        def build_b(gram_src):
            with (
                tc.tile_pool(name="bb", bufs=2) as bp,
                tc.tile_pool(name="bb_ps", bufs=2, space="PSUM") as bpp,
            ):
                dsq = bp.tile([P, NAB], F32, tag="bb_dsq")
                for t in range(NAB):
                    gd = bp.tile([P, P], F32, tag="bb_gd")
                    nc.sync.dma_start(gd, gram_src[0, ts(t, P), ts(t, P)])
                    nc.vector.tensor_mul(gd, gd, identity)
                    nc.vector.tensor_reduce(
                        dsq[:, ds(t, 1)], gd, mybir.AxisListType.X,
                        mybir.AluOpType.add)
                ninv2 = bp.tile([P, NAB], F32, tag="bb_ninv2")
                ninv = bp.tile([P, NAB], F32, tag="bb_ninv")
                w = bp.tile([P, NAB], F32, tag="bb_w")
                nc.vector.tensor_scalar_add(ninv2, dsq, EPS)
                nc.vector.reciprocal(ninv2, ninv2)
                nc.scalar.sqrt(ninv, ninv2)
                nc.vector.tensor_scalar_mul(w, ninv2, -0.5)
                # vrow [1, R]: ninv flattened in column order
                vT_ps = bpp.tile([NAB, P], F32, tag="bb_vT")
                nc.tensor.transpose(vT_ps, ninv, identity)
                vT = bp.tile([NAB, P], F32, tag="bb_vTs")
                nc.vector.tensor_copy(vT, vT_ps)
                vrow = persist.tile([1, R], F32, tag="vrow", name="vrow")
                nc.sync.dma_start(vrow, vT)
                vfull = bp.tile([P, R], F32, tag="bb_vfull")
                bcast_row(vrow, vfull, bpp, "bb_vps")
                for c in range(2):
                    for t in range(NAB):
                        bg = bp.tile([P, R], F32, tag="bb_bg")
                        nc.sync.dma_start(bg, gram_src[c, ts(t, P), :])
                        nc.vector.tensor_scalar_mul(bg, bg, w[:, ds(t, 1)])
                        nc.vector.tensor_mul(bg, bg, vfull)
                        bd = bg[:, ts(t, P)]
                        nc.vector.tensor_mul(bd, bd, omI)
                        if c == 0:
                            dg = bp.tile([P, P], F32, tag="bb_dg")
                            nc.vector.tensor_scalar_mul(dg, identity,
                                                        ninv[:, ds(t, 1)])
                            nc.vector.tensor_add(bd, bd, dg)
                        nc.vector.tensor_copy(b_t[c][t], bg)
                for t in range(NAB):
                    nc.vector.tensor_add(b_t[2][t], b_t[0][t], b_t[1][t])

        # ---------------- update pass ----------------
        def update_pass(load_rows, q_dst, acc_sq):
            """Q_next[m] = Q'[m] @ B via Karatsuba (3 real matmul products);
            optionally accumulate per-column sum of squares into acc_sq [P, R]
            (still needs a cross-partition reduce afterwards)."""
            with (
                tc.tile_pool(name="up_ld", bufs=2) as ldp,
                tc.tile_pool(name="up_t", bufs=2) as tp_sb,
                tc.tile_pool(name="up_ps", bufs=2, space="PSUM") as tpp,
                tc.tile_pool(name="up_ops", bufs=1, space="PSUM") as opp,
                tc.tile_pool(name="up_out", bufs=2) as outp,
            ):
                for m in range(NMT):
                    qr_t, qi_t = load_rows(ldp, m)
                    qrT = tp_sb.tile([P, NAB, P], BF16, tag="qrT")
                    qiT = tp_sb.tile([P, NAB, P], BF16, tag="qiT")
                    qsT = tp_sb.tile([P, NAB, P], BF16, tag="qsT")
                    for src, dstT in ((qr_t, qrT), (qi_t, qiT)):
                        for g in range(2):
                            tp = tpp.tile([P, 4, P], BF16, tag="tp",
                                          name="tp")
                            for k in range(4):
                                nc.tensor.transpose(tp[:, k, :],
                                                    src[:, ts(4 * g + k, P)],
                                                    identity_bf)
                            nc.scalar.copy(dstT[:, ds(4 * g, 4), :], tp)
                    nc.vector.tensor_add(qsT, qrT, qiT)
                    p1 = opp.tile([P, R], F32, tag="p1", name="p1")
                    p2 = opp.tile([P, R], F32, tag="p2", name="p2")
                    p3 = opp.tile([P, R], F32, tag="p3", name="p3")
                    for n in range(NCH):
                        nsl = ds(n * CW, CW)
                        for ps, qT, bc in ((p1, qrT, 0), (p2, qiT, 1),
                                           (p3, qsT, 2)):
                            for a in range(NAB):
                                nc.tensor.matmul(ps[:, nsl], qT[:, a, :],
                                                 b_t[bc][a][:, nsl],
                                                 start=(a == 0),
                                                 stop=(a == NAB - 1))
                    qn = outp.tile([P, 2, R], BF16, tag="qn", name="qn")
                    qf = outp.tile([P, 2, R], F32, tag="qf", name="qf")
                    nc.vector.tensor_copy(qf[:, 0, :], p1)
                    nc.scalar.copy(qf[:, 1, :], p3)
                    nc.vector.tensor_sub(qf[:, 1, :], qf[:, 1, :], p1)
                    nc.vector.tensor_sub(qn[:, 1, :], qf[:, 1, :], p2)
                    nc.vector.tensor_sub(qn[:, 0, :], qf[:, 0, :], p2)
                    if acc_sq is not None:
                        sq = ldp.tile([P, 2, R], F32, tag="sq", name="sq")
                        nc.vector.tensor_mul(sq, qn, qn)
                        if m == 0:
                            nc.vector.tensor_add(acc_sq, sq[:, 0, :],
                                                 sq[:, 1, :])
                        else:
                            nc.vector.tensor_add(acc_sq, acc_sq, sq[:, 0, :])
                            nc.vector.tensor_add(acc_sq, acc_sq, sq[:, 1, :])
                    nc.sync.dma_start(q_dst[ts(m, P)], qn)

        # ---------------- loaders ----------------
        def load_cols_x(pool, m, ch):
            xt = pool.tile([P, CW, 2], F32, tag="xcols")
            nc.sync.dma_start(xt, x[ts(m, P), ds(ch * CW, CW), :])
            qr = pool.tile([P, CW], BF16, tag="xc_r")
            qi = pool.tile([P, CW], BF16, tag="xc_i")
            nc.vector.tensor_copy(qr, xt[:, :, 0])
            nc.vector.tensor_copy(qi, xt[:, :, 1])
            return qr, qi

        def load_cols_q1(pool, m, ch):
            qt = pool.tile([P, 2, CW], BF16, tag="xcols")
            nc.sync.dma_start(qt, q1[ts(m, P), :, ds(ch * CW, CW)])
            return qt[:, 0, :], qt[:, 1, :]

        def load_rows_x(pool, m):
            xt = pool.tile([P, R, 2], F32, tag="xrows")
            nc.sync.dma_start(xt, x[ts(m, P), :, :])
            qr = pool.tile([P, R], BF16, tag="xr_r")
            qi = pool.tile([P, R], BF16, tag="xr_i")
            nc.vector.tensor_copy(qr, xt[:, :, 0])
            nc.vector.tensor_copy(qi, xt[:, :, 1])
            return qr, qi

        def load_rows_q1(pool, m):
            qt = pool.tile([P, 2, R], BF16, tag="xrows")
            nc.sync.dma_start(qt, q1[ts(m, P)])
            return qt[:, 0, :], qt[:, 1, :]

        rg = [list(range(N_CORES))]

        def all_reduce(dst, src):
            if single_core:
                nc.sync.dma_start(dst[:], src[:])
            else:
                nc.gpsimd.collective_compute(
                    "AllReduce", mybir.AluOpType.add, replica_groups=rg,
                    ins=[src[:]], outs=[dst[:]])

        def debug_out():
            """Write gram_in[0] head into out so every stage has output."""
            with tc.tile_pool(name="dbg", bufs=2) as dp:
                for m in range(NMT):
                    t = dp.tile([P, R, 2], F32, tag="dbg_t")
                    nc.vector.memset(t, 0.0)
                    nc.sync.dma_start(out[ts(m, P), :, :], t)

        def _one_rep(rep):
            gram_in = [dram.tile([2, R, R], F32, tag=f"gin{k}_{rep}",
                                 name=f"gin{k}_{rep}") for k in range(2)]
            gram_out = [dram.tile([2, R, R], F32, tag=f"gout{k}_{rep}",
                                  name=f"gout{k}_{rep}", addr_space="Shared")
                        for k in range(2)]
            dsq_in = dram.tile([P, NAB], F32, tag=f"dsq_in_{rep}",
                               name=f"dsq_in_{rep}")
            dsq_out = dram.tile([P, NAB], F32, tag=f"dsq_out_{rep}",
                                name=f"dsq_out_{rep}", addr_space="Shared")
            gram_pass(load_cols_x, gram_in[0])
            if debug_stage >= 2:
                all_reduce(gram_out[0], gram_in[0])
            if debug_stage >= 3:
                build_b(gram_out[0])
            if debug_stage >= 4:
                update_pass(load_rows_x, q1, None)

            # ======== iteration 1 ========
            if debug_stage >= 5:
                gram_pass(load_cols_q1, gram_in[1])
                all_reduce(gram_out[1], gram_in[1])
                build_b(gram_out[1])
            def _full_tail():
                acc_sq = persist.tile([P, R], F32, tag="acc_sq",
                                      name="acc_sq")
                update_pass(load_rows_q1, q2, acc_sq)

                # ======== final column norms ========
                with (
                    tc.tile_pool(name="fn", bufs=2) as fp,
                    tc.tile_pool(name="fn_ps", bufs=2, space="PSUM") as fpp,
                ):
                    # cross-partition reduce of acc_sq via ones-matmul
                    dsq_row = fp.tile([1, R], F32, tag="fn_dsqrow")
                    onesc = fp.tile([P, 1], F32, tag="fn_onesc")
                    nc.vector.memset(onesc, 1.0)
                    for n in range(NCH):
                        nsl = ds(n * CW, CW)
                        rp = fpp.tile([1, CW], F32, tag="fn_rp", name="fn_rp")
                        nc.tensor.matmul(rp, onesc, acc_sq[:, nsl],
                                         start=True, stop=True)
                        nc.vector.tensor_copy(dsq_row[:, nsl], rp)
                    nc.sync.dma_start(dsq_in, dsq_row)
                    all_reduce(dsq_out, dsq_in)
                    srow = fp.tile([1, R], F32, tag="fn_srow")
                    nc.sync.dma_start(srow, dsq_out)
                    nc.vector.tensor_scalar_add(srow, srow, EPS)
                    nc.vector.reciprocal(srow, srow)
                    nc.scalar.sqrt(srow, srow)
                    sfull = persist.tile([P, 2, R], BF16, tag="sfull",
                                         name="sfull")
                    for n in range(NCH):
                        nsl = ds(n * CW, CW)
                        v_ps = fpp.tile([P, CW], F32, tag="fn_sps",
                                        name="fn_sps")
                        nc.tensor.matmul(v_ps, ones1, srow[:, nsl],
                                         start=True, stop=True)
                        nc.vector.tensor_copy(sfull[:, 0, nsl], v_ps)
                        nc.vector.tensor_copy(sfull[:, 1, nsl], v_ps)

                # ======== final scale + interleave store ========
                with tc.tile_pool(name="st", bufs=3) as sp:
                    for m in range(NMT):
                        qt = sp.tile([P, 2, R], BF16, tag="st_q")
                        nc.sync.dma_start(qt, q2[ts(m, P)])
                        nc.vector.tensor_mul(qt, qt, sfull)
                        ot = sp.tile([P, R, 2], F32, tag="st_o")
                        nc.vector.tensor_copy(ot[:, :, 0], qt[:, 0, :])
                        nc.vector.tensor_copy(ot[:, :, 1], qt[:, 1, :])
                        nc.sync.dma_start(out[ts(m, P), :, :], ot)

            if debug_stage < 6:
                debug_out()
            else:
                _full_tail()

        for _rep in range(reps):
            _one_rep(_rep)

    _finalize(nc)
    return nc


def kernel(x: np.ndarray) -> np.ndarray:
    assert x.shape == (M_FULL, R, 2) and x.dtype == np.float32
    if "nc" not in _CACHE:
        _CACHE["nc"] = _build_nc()
    nc = _CACHE["nc"]
    in_maps = [{"x": np.ascontiguousarray(x[i * MS:(i + 1) * MS])}
               for i in range(N_CORES)]
    res = run_bass_kernel_spmd(nc, in_maps, core_ids=list(range(N_CORES)))
    return np.concatenate([res.results[i]["out"] for i in range(N_CORES)],
                          axis=0)

